# revision 14
# baseline (speedup 1.0000x reference)
"""Trainium2 Bass kernel for nn_AttentionHead (B=8, T=512, V=25, C=128, Dk=Dv=64).

Sharding: data-parallel over batch B across 8 NeuronCores (batch b -> core b).
No cross-device communication.

Per-core design (per vertex v, fp16 datapath, fp32 PSUM accumulation):
- x pre-transposed on host to (V, C, T): channel contraction lands on SBUF
  partitions with fully contiguous DMA; one DMA in, one DMA out per vertex.
- scores computed transposed ([s, t]) per 128-row s-chunk -> no on-chip
  transposes anywhere. The 4 triangular chunks (512/384/256/128 valid t cols)
  are packed into 2.5 PSUM banks at column bases [0, 512, 1024, 896] so ONE
  exp activation instruction covers exactly the 1280 valid columns.
- causal masking: only the diagonal 128x128 block of each chunk needs a mask;
  it is applied *after* exp by zeroing the lower triangle of E with
  affine_select on the (otherwise idle) GPSIMD engine.
- the value matrix is augmented with a ones-column, so the out matmul
  accumulates the softmax denominator into [t, 1] orientation for free.
- reference semantics: unoccupied slots (s > t) contribute exp(0)=1 to the
  denominator: denominator = sum_{s<=t} exp(scores) + (511 - t), added via a
  host-precomputed count table.
"""

import numpy as np
from contextlib import ExitStack

import concourse.bass as bass  # noqa: F401
import concourse.tile as tile
from concourse import bacc, mybir
from concourse.bass_utils import run_bass_kernel_spmd

B, T, V, C = 8, 512, 25, 128
DK, DV = 64, 64
P = 128
NT = T // P  # 4 tiles of 128 along T
N_CORES = 8

# packed column bases of the 4 score chunks inside the 3-bank PSUM tile
SB_BASE = [0, 512, 1024, 896]
SB_NJ = [T - j * P for j in range(NT)]  # 512, 384, 256, 128
SC_TOT = 1280  # total valid score columns

F32 = mybir.dt.float32
F16 = mybir.dt.float16
AF = mybir.ActivationFunctionType
ALU = mybir.AluOpType

_PROGRAM_CACHE = {}


def build_program(n_v=V, n_rep=1):
    nc = bacc.Bacc(
        "TRN2", target_bir_lowering=False, debug=False, num_devices=N_CORES
    )
    xt = nc.dram_tensor("xt", [n_v, C, T], F16, kind="ExternalInput").ap()
    wq = nc.dram_tensor("wq", [C, DK], F16, kind="ExternalInput").ap()
    wk = nc.dram_tensor("wk", [C, DK], F16, kind="ExternalInput").ap()
    wv = nc.dram_tensor("wv", [C, DK], F16, kind="ExternalInput").ap()
    bqk = nc.dram_tensor("bqk", [DK, 2], F32, kind="ExternalInput").ap()
    bvb = nc.dram_tensor("bvb", [P, NT * DV], F32, kind="ExternalInput").ap()
    cnt = nc.dram_tensor("cnt", [P, NT], F32, kind="ExternalInput").ap()
    out = nc.dram_tensor("out", [n_v, T, DV], F32, kind="ExternalOutput").ap()

    with tile.TileContext(nc) as tc, ExitStack() as ctx:
        consts = ctx.enter_context(tc.tile_pool(name="consts", bufs=1))
        sbx = ctx.enter_context(tc.tile_pool(name="sbx", bufs=3))
        sbqk = ctx.enter_context(tc.tile_pool(name="sbqk", bufs=3))
        sbv = ctx.enter_context(tc.tile_pool(name="sbv", bufs=3))
        sbe = ctx.enter_context(tc.tile_pool(name="sbe", bufs=2))
        sbo = ctx.enter_context(tc.tile_pool(name="sbo", bufs=3))
        sbs = ctx.enter_context(tc.tile_pool(name="sbs", bufs=4))
        psq = ctx.enter_context(tc.tile_pool(name="psq", bufs=1, space="PSUM"))
        psv = ctx.enter_context(tc.tile_pool(name="psv", bufs=1, space="PSUM"))
        pso = ctx.enter_context(tc.tile_pool(name="pso", bufs=2, space="PSUM"))
        pss = ctx.enter_context(tc.tile_pool(name="pss", bufs=1, space="PSUM"))

        wq_t = consts.tile([C, DK], F16)
        nc.sync.dma_start(wq_t[:], wq[:])
        wk_t = consts.tile([C, DK], F16)
        nc.sync.dma_start(wk_t[:], wk[:])
        wv_t = consts.tile([C, DK], F16)
        nc.sync.dma_start(wv_t[:], wv[:])
        bqk_t = consts.tile([DK, 2], F32)
        nc.sync.dma_start(bqk_t[:], bqk[:])
        bvb_t = consts.tile([P, NT * DV], F32)
        nc.sync.dma_start(bvb_t[:], bvb[:])
        cnt_t = consts.tile([P, NT], F32)
        nc.sync.dma_start(cnt_t[:], cnt[:])

        for rep in range(n_rep):
          xt_pair = {}
          of_pair = {}
          state = {}

          def front(v):
            if v % 2 == 0:
                xp = sbx.tile([C, 2, T], F16, tag="xt", name="xp")
                hi = min(2, n_v - v)
                nc.sync.dma_start(
                    xp[:, 0:hi, :],
                    xt[v:v + hi].rearrange("v c t -> c v t"))
                xt_pair[v] = xp
            xt_t = xt_pair[v - v % 2][:, v % 2, :]

            qk_ps = psq.tile([DK, 2 * T], F32, tag="qk", name="qk_ps")
            nc.tensor.matmul(qk_ps[:, 0:T], wq_t[:], xt_t,
                             start=True, stop=True)
            nc.tensor.matmul(qk_ps[:, T:2 * T], wk_t[:], xt_t,
                             start=True, stop=True)
            v_ps = psv.tile([P, NT * DV], F32, tag="vp", name="v_ps")
            for j in range(NT):
                nc.tensor.matmul(v_ps[:, j * DV:(j + 1) * DV],
                                 xt_t[:, j * P:(j + 1) * P],
                                 wv_t[:], start=True, stop=True)

            qk_sb = sbqk.tile([DK, 2 * T], F16, tag="qks", name="qk_sb")
            nc.vector.tensor_add(
                qk_sb[:].rearrange("p (h t) -> p h t", h=2),
                qk_ps[:].rearrange("p (h t) -> p h t", h=2),
                bqk_t[:, :, None].broadcast_to([DK, 2, T]))
            qt_sb = qk_sb[:, 0:T]
            kt_sb = qk_sb[:, T:2 * T]

            v4_sb = sbv.tile([P, NT * (DV + 1)], F16, tag="v4", name="v4_sb")
            v4 = v4_sb[:].rearrange("p (c e) -> p c e", e=DV + 1)
            vp4 = v_ps[:].rearrange("p (c e) -> p c e", e=DV)
            nc.vector.tensor_add(v4[:, :, 0:DV], vp4[:], bvb_t[:])
            nc.gpsimd.memset(v4[:, :, DV:DV + 1], 1.0)
            state[v] = (qt_sb, kt_sb, v4_sb)

          def mid(v):
            qt_sb, kt_sb, v4_sb = state[v]
            s_ps = pss.tile([P, 3 * 512], F32, tag="sct", name="s_ps")
            for j in range(NT):
                bj, nj = SB_BASE[j], SB_NJ[j]
                nc.tensor.matmul(s_ps[:, bj:bj + nj],
                                 kt_sb[:, j * P:(j + 1) * P],
                                 qt_sb[:, j * P:T],
                                 start=True, stop=True)
            et = sbe.tile([P, SC_TOT], F16, tag="et", name="et")
            nc.scalar.activation(et[:], s_ps[:, 0:SC_TOT], AF.Exp)
            for j in range(NT):
                bj = SB_BASE[j]
                nc.gpsimd.affine_select(
                    out=et[:, bj:bj + P], in_=et[:, bj:bj + P],
                    compare_op=ALU.is_ge, fill=0.0,
                    base=0, pattern=[[1, P]], channel_multiplier=-1)
            state[v] = (v4_sb, et)

          def back(v):
            v4_sb, et = state.pop(v)
            o_ps = pso.tile([P, NT * (DV + 1)], F32, tag="op", name="o_ps")
            o4 = o_ps[:].rearrange("p (i e) -> p i e", e=DV + 1)
            for i in range(NT):
                for j in range(i + 1):
                    nc.tensor.matmul(
                        o_ps[:, i * (DV + 1):(i + 1) * (DV + 1)],
                        et[:, SB_BASE[j] + (i - j) * P:
                               SB_BASE[j] + (i - j + 1) * P],
                        v4_sb[:, j * (DV + 1):(j + 1) * (DV + 1)],
                        start=(j == 0), stop=(j == i))

            den4 = sbs.tile([P, NT], F32, tag="den", name="den4")
            nc.vector.tensor_add(den4[:], o4[:, :, DV:DV + 1].rearrange(
                "p i e -> p (i e)"), cnt_t[:])
            rec4 = sbs.tile([P, NT], F32, tag="rec", name="rec4")
            nc.vector.reciprocal(rec4[:], den4[:])
            if v % 2 == 0:
                of_pair[v] = sbo.tile([P, 2, NT * DV], F32, tag="of",
                                      name="ofp")
            ofp = of_pair[v - v % 2]
            of4v = ofp[:, v % 2, :].rearrange("p (i e) -> p i e", e=DV)
            nc.vector.tensor_mul(of4v[:], o4[:, :, 0:DV],
                                 rec4[:, :, None].broadcast_to([P, NT, DV]))
            if v % 2 == 1 or v == n_v - 1:
                v0 = v - v % 2
                hi = v % 2 + 1
                nc.sync.dma_start(
                    out[v0:v0 + hi].rearrange("v (i p) e -> p v i e", p=P),
                    of_pair.pop(v0)[:, 0:hi, :].rearrange(
                        "p v (i e) -> p v i e", e=DV))

          for k in range(n_v + 2):
            if k < n_v:
                front(k)
            if 0 <= k - 1 < n_v:
                mid(k - 1)
            if 0 <= k - 2 < n_v:
                back(k - 2)

    nc.compile()
    return nc


def get_program(n_v=V, n_rep=1):
    key = (n_v, n_rep)
    if key not in _PROGRAM_CACHE:
        _PROGRAM_CACHE[key] = build_program(n_v, n_rep)
    return _PROGRAM_CACHE[key]


def host_inputs(x, Wq, bq, Wk, bk, Wv, bv):
    """Build the per-core input maps (host-side data staging)."""
    x = np.asarray(x, dtype=np.float32)
    Wq = np.asarray(Wq, dtype=np.float32)
    bq = np.asarray(bq, dtype=np.float32)
    Wk = np.asarray(Wk, dtype=np.float32)
    bk = np.asarray(bk, dtype=np.float32)
    Wv = np.asarray(Wv, dtype=np.float32)
    bv = np.asarray(bv, dtype=np.float32)

    scale = np.float32(1.0 / np.sqrt(np.float32(DK)))
    wqh = np.ascontiguousarray((Wq * scale).T).astype(np.float16)  # (C, DK)
    bqh = (bq * scale).reshape(DK, 1)
    wkh = np.ascontiguousarray(Wk.T).astype(np.float16)            # (C, DK)
    bkh = bk.reshape(DK, 1)
    bqkh = np.ascontiguousarray(np.concatenate([bqh, bkh], axis=1)).astype(np.float32)
    wvh = np.ascontiguousarray(Wv.T).astype(np.float16)            # (C, DV)
    bvbh = np.ascontiguousarray(
        np.broadcast_to(bv, (P, NT, DV)).reshape(P, NT * DV)).astype(np.float32)

    tl = np.arange(P, dtype=np.int64)
    ii = np.arange(NT, dtype=np.int64)
    cnth = ((T - 1) - (ii[None, :] * P + tl[:, None])).astype(np.float32)

    # (B, T, V, C) -> (B, V, C, T), fp16
    xth = np.ascontiguousarray(x.transpose(0, 2, 3, 1)).astype(np.float16)

    in_maps = []
    for b in range(N_CORES):
        in_maps.append({
            "xt": xth[b],
            "wq": wqh, "wk": wkh, "wv": wvh,
            "bqk": bqkh, "bvb": bvbh,
            "cnt": cnth,
        })
    return in_maps


def run(x, Wq, bq, Wk, bk, Wv, bv, trace=False):
    """Run on 8 cores; returns (output, BassKernelResults)."""
    nc = get_program(V)
    in_maps = host_inputs(x, Wq, bq, Wk, bk, Wv, bv)
    res = run_bass_kernel_spmd(nc, in_maps, list(range(N_CORES)), trace=trace)
    outp = np.empty((B, T, V, DV), dtype=np.float32)
    for b in range(N_CORES):
        outp[b] = res.results[b]["out"].transpose(1, 0, 2)
    return outp, res


def kernel(x, Wq, bq, Wk, bk, Wv, bv):
    outp, _ = run(x, Wq, bq, Wk, bk, Wv, bv, trace=False)
    return outp
